# revision 8
# baseline (speedup 1.0000x reference)
"""Trainium2 Bass kernel for nn_DeepMambaModel (decision-transformer-style Mamba).

Sharding: 8 cores = 4 batch groups x 2 d_inner halves (Megatron-style TP on the
Mamba inner dim).  Each pair of cores (2b, 2b+1) handles batch b; within the
pair the selective-scan region (the elementwise-heavy (T, DI, N) work) is split
over d_inner halves, the small matmuls (in_proj/conv/x_proj) are replicated,
and the only cross-core traffic is one pairwise AllGather of the gated scan
output per layer.

Layout: activations live transposed on-chip as [feature-partition, token-free]
tiles so every matmul is lhsT.T @ rhs with K on partitions.  The causal
depthwise conv is folded into in_proj as 4 shifted accumulating matmuls.
dA = exp(A[n] * delta) is produced by ScalarE exp-with-scale for n=0..7 and by
exact exponent-addition products on VectorE for n=8..15 (c_n = n+1 here).  The
scan itself is tensor_tensor_scan (state = dA*state + dBu) along tokens.
All transcendentals (softplus, silu, rsqrt) are built from exp/ln so a single
ACT table set is used for the whole kernel.

Per-core channel permutation: each core's d_inner channels are ordered
[own half, other half] in its shipped weights, so the own half is always block
0..NBLK-1 of u/xc with an identical program on every core; the AllGather output
is in original order (rank0 half first), so out_w ships unpermuted.
"""

import numpy as np

import concourse.bass as bass
import concourse.bacc as bacc
import concourse.mybir as mybir
import concourse.tile as tile
from concourse.bass_utils import run_bass_kernel_spmd

# ---------------------------------------------------------------- dimensions
B, L, D = 4, 64, 512
DI, N, DC, R = 1024, 16, 4, 32
NL = 4
SD, AD, MEL = 17, 6, 1000
EPS = 1e-5

T = 4 * L                  # 256 interleaved tokens
HD = DI // 2               # d_inner half per core
KT = D // 128              # 4   k-tiles over d_model
PBLK = D // 128            # 4   partition blocks of the residual stream
NBLK = HD // 128           # 4   partition blocks of the owned d_inner half
FBLK = DI // 128           # 8   partition blocks of full d_inner
RN = R + 2 * N             # 64  x_proj output channels

f16 = mybir.dt.float16
f32 = mybir.dt.float32
AOP = mybir.AluOpType
AFT = mybir.ActivationFunctionType

# exponent-addition pairs: dA[n] = dA[a] * dA[b]  (valid when c_n ~ n+1)
POW_PAIRS = {8: (3, 4), 9: (4, 4), 10: (4, 5), 11: (5, 5),
             12: (5, 6), 13: (6, 6), 14: (6, 7), 15: (7, 7)}


# =============================================================== program
def build_program(n_pairs: int, dve_powers: bool = True) -> bass.Bass:
    nc = bacc.Bacc()
    # register the extra float constant used as an activation bias (eps)
    _ct = nc.alloc_sbuf_tensor(f"const-f32-eps", [128, 1], f32)
    nc.gpsimd.memset(_ct.ap(), EPS)
    nc.const_aps.aps[(f32, EPS)] = _ct.ap()
    nc.all_engine_barrier()
    dp = nc.declare_dram_parameter

    i_sT = dp("i_sT", [SD, L], f16, isOutput=False)
    i_aT = dp("i_aT", [AD, L], f16, isOutput=False)
    i_rcT = dp("i_rcT", [1, 2 * L], f16, isOutput=False)
    i_teT = dp("i_teT", [128, PBLK, L], f16, isOutput=False)
    i_wes = dp("i_wes", [SD, D], f16, isOutput=False)
    i_wea = dp("i_wea", [AD, D], f16, isOutput=False)
    i_werc = dp("i_werc", [1, 2 * D], f16, isOutput=False)
    i_bemb = dp("i_bemb", [1, 4 * D], f16, isOutput=False)
    i_ones = dp("i_ones", [1, T], f16, isOutput=False)
    i_onesp = dp("i_onesp", [1, 128], f16, isOutput=False)
    i_wstat = dp("i_wstat", [128, 1], f16, isOutput=False)
    i_lnw = dp("i_lnw", [128, NL + 1, PBLK], f32, isOutput=False)
    i_lnb = dp("i_lnb", [128, NL + 1, PBLK], f32, isOutput=False)
    i_asc = dp("i_asc", [128, NL, NBLK, N], f32, isOutput=False)
    i_dpr = dp("i_dpr", [128, NL, NBLK], f32, isOutput=False)
    i_dtb = dp("i_dtb", [128, NL, NBLK], f32, isOutput=False)
    i_wps = dp("i_wps", [128, PBLK, SD], f16, isOutput=False)
    i_wpa = dp("i_wpa", [128, PBLK, AD], f16, isOutput=False)
    i_bps = dp("i_bps", [1, SD], f16, isOutput=False)
    i_bpa = dp("i_bpa", [1, AD], f16, isOutput=False)
    i_win = dp("i_win", [NL, DC, 128, KT, DI], f16, isOutput=False)
    i_wz = dp("i_wz", [NL, 128, KT, HD], f16, isOutput=False)
    i_wxp = dp("i_wxp", [NL, 128, FBLK, RN], f16, isOutput=False)
    i_wdt = dp("i_wdt", [NL, 32, HD], f16, isOutput=False)
    i_bcv = dp("i_bcv", [NL, 1, DI], f16, isOutput=False)
    i_wout = dp("i_wout", [NL, 128, FBLK, D], f16, isOutput=False)

    o_sp = dp("o_spT", [SD, L], f32, isOutput=True)
    o_ap = dp("o_apT", [AD, L], f32, isOutput=True)

    groups = [[2 * i, 2 * i + 1] for i in range(n_pairs)]

    from contextlib import ExitStack

    with tile.TileContext(nc) as tc, ExitStack() as es:
        cpool = es.enter_context(tc.tile_pool(name="consts", bufs=1))
        wpool = es.enter_context(tc.tile_pool(name="weights", bufs=2))
        apool = es.enter_context(tc.tile_pool(name="acts", bufs=1))
        spool = es.enter_context(tc.tile_pool(name="scan", bufs=2))
        s1pool = es.enter_context(tc.tile_pool(name="scan1", bufs=1))
        pp = es.enter_context(tc.tile_pool(name="ps", bufs=1, space="PSUM"))
        dpool = es.enter_context(tc.tile_pool(name="drampool", bufs=1, space="DRAM"))

        # ---- persistent tiles / constants
        xTh = cpool.tile([128, PBLK, T], f16, name="xTh")     # residual stream
        ones_r = cpool.tile([1, T], f16, name="ones_r")
        ones_p = cpool.tile([1, 128], f16, name="ones_p")
        wstat = cpool.tile([128, 1], f16, name="wstat")
        lnw = cpool.tile([128, NL + 1, PBLK], f32, name="lnw")
        lnb = cpool.tile([128, NL + 1, PBLK], f32, name="lnb")
        asc = cpool.tile([128, NL, NBLK, N], f32, name="asc")
        dpr = cpool.tile([128, NL, NBLK], f32, name="dpr")
        dtb = cpool.tile([128, NL, NBLK], f32, name="dtb")
        teT = cpool.tile([128, PBLK, L], f16, name="teT")

        nc.sync.dma_start(ones_r[:], i_ones[:])
        nc.sync.dma_start(ones_p[:], i_onesp[:])
        nc.sync.dma_start(wstat[:], i_wstat[:])
        nc.sync.dma_start(lnw[:], i_lnw[:])
        nc.sync.dma_start(lnb[:], i_lnb[:])
        nc.sync.dma_start(asc[:], i_asc[:])
        nc.sync.dma_start(dpr[:], i_dpr[:])
        nc.sync.dma_start(dtb[:], i_dtb[:])
        nc.sync.dma_start(teT[:], i_teT[:])

        # ================================================= embeddings
        w_es = cpool.tile([SD, D], f16, name="w_es")
        w_ea = cpool.tile([AD, D], f16, name="w_ea")
        w_erc = cpool.tile([1, 2 * D], f16, name="w_erc")
        b_emb = cpool.tile([1, 4 * D], f16, name="b_emb")
        sT = cpool.tile([SD, L], f16, name="sT")
        aT = cpool.tile([AD, L], f16, name="aT")
        rcT = cpool.tile([1, 2 * L], f16, name="rcT")
        nc.sync.dma_start(w_es[:], i_wes[:])
        nc.sync.dma_start(w_ea[:], i_wea[:])
        nc.sync.dma_start(w_erc[:], i_werc[:])
        nc.sync.dma_start(b_emb[:], i_bemb[:])
        nc.sync.dma_start(sT[:], i_sT[:])
        nc.sync.dma_start(aT[:], i_aT[:])
        nc.sync.dma_start(rcT[:], i_rcT[:])

        # token order k = 0:return 1:constraint 2:state 3:action
        xTh_v = xTh[:].rearrange("p b (l k) -> p b l k", k=4)
        for pb in range(PBLK):
            csl = slice(pb * 128, (pb + 1) * 128)
            streams = [
                (w_erc[:, pb * 128:pb * 128 + 128], rcT[:, 0:L], 2),
                (w_erc[:, D + pb * 128:D + pb * 128 + 128], rcT[:, L:2 * L], 3),
                (w_es[:, csl], sT[:], 0),
                (w_ea[:, csl], aT[:], 1),
            ]
            for k, (wT, rhs, brow) in enumerate(streams):
                e_ps = pp.tile([128, L], f32, tag=("s1" if k % 2 == 0 else "s2"),
                               name="e_ps")
                nc.tensor.matmul(e_ps[:], wT, rhs, start=True, stop=False)
                nc.tensor.matmul(
                    e_ps[:], b_emb[:, brow * D + pb * 128:brow * D + pb * 128 + 128],
                    ones_r[:, 0:L], start=False, stop=True)
                nc.vector.tensor_add(xTh_v[:, pb, :, k], e_ps[:], teT[:, pb, :])

        # ================================================= layernorm helper
        def layer_norm(l_idx, out_xhat):
            sq = apool.tile([128, PBLK, T], f16, tag="ln_sq", name="sq")
            nc.scalar.activation(sq[:], xTh[:], AFT.Square)
            mu_ps = pp.tile([1, T], f32, tag="s1", name="mu_ps")
            ex_ps = pp.tile([1, T], f32, tag="s2", name="ex_ps")
            for pb in range(PBLK):
                nc.tensor.matmul(mu_ps[:], wstat[:], xTh[:, pb, :],
                                 start=(pb == 0), stop=(pb == PBLK - 1))
            for pb in range(PBLK):
                nc.tensor.matmul(ex_ps[:], wstat[:], sq[:, pb, :],
                                 start=(pb == 0), stop=(pb == PBLK - 1))
            musq = apool.tile([1, T], f32, tag="ln_s1", name="musq")
            nc.scalar.activation(musq[:], mu_ps[:], AFT.Square)
            mu_s = apool.tile([1, T], f16, tag="ln_s2", name="mu_s")
            nc.scalar.copy(mu_s[:], mu_ps[:])
            var_s = apool.tile([1, T], f32, tag="ln_s3", name="var_s")
            nc.vector.tensor_tensor(var_s[:], ex_ps[:], musq[:], AOP.subtract)
            lv = apool.tile([1, T], f32, tag="ln_s4", name="lv")
            nc.scalar.activation(lv[:], var_s[:], AFT.Ln, bias=EPS)
            rstd = apool.tile([1, T], f16, tag="ln_s5", name="rstd")
            nc.scalar.activation(rstd[:], lv[:], AFT.Exp, scale=-0.5)
            mur_ps = pp.tile([128, T], f32, tag="s1", name="mur_ps")
            rsr_ps = pp.tile([128, T], f32, tag="s2", name="rsr_ps")
            nc.tensor.matmul(mur_ps[:], ones_p[:], mu_s[:], start=True, stop=True)
            nc.tensor.matmul(rsr_ps[:], ones_p[:], rstd[:], start=True, stop=True)
            for pb in range(PBLK):
                cen = apool.tile([128, T], f32, tag="ln_cen", name="cen")
                nc.vector.tensor_tensor(cen[:], xTh[:, pb, :], mur_ps[:],
                                        AOP.subtract)
                pre = apool.tile([128, T], f32, tag="ln_pre", name="pre")
                nc.vector.tensor_tensor(pre[:], cen[:], rsr_ps[:], AOP.mult)
                nc.vector.scalar_tensor_tensor(
                    out_xhat[:, pb, :], pre[:], lnw[:, l_idx, pb:pb + 1],
                    lnb[:, l_idx, pb:pb + 1].broadcast_to([128, T]),
                    op0=AOP.mult, op1=AOP.add)

        # ================================================= layers
        for l in range(NL):
            xhat = apool.tile([128, PBLK, T], f16, tag="xhat", name="xhat")
            layer_norm(l, xhat)

            # ---- in_proj with folded causal conv -> xc_ps;  z branch
            xc_ps = pp.tile([128, FBLK, T], f32, tag="big", name="xc_ps")
            w_cv = []
            for k in range(DC):
                w_cvk = wpool.tile([128, KT, DI], f16, tag=f"w_cv{k}",
                                   name=f"w_cv{k}", bufs=1)
                nc.sync.dma_start(w_cvk[:], i_win[l, k])
                w_cv.append(w_cvk)
            bcv = wpool.tile([1, DI], f16, tag="bcv", name="bcv")
            nc.sync.dma_start(bcv[:], i_bcv[l])
            # one PSUM accumulation group per mt (one open group per bank)
            for mt in range(FBLK):
                for k in (3, 2, 1, 0):
                    lo = 3 - k      # first valid output token for this shift
                    for kt in range(KT):
                        nc.tensor.matmul(
                            xc_ps[:, mt, lo:T],
                            w_cv[k][:, kt, mt * 128:(mt + 1) * 128],
                            xhat[:, kt, 0:T - lo],
                            start=(k == 3 and kt == 0), stop=False)
                nc.tensor.matmul(xc_ps[:, mt, :],
                                 bcv[:, mt * 128:(mt + 1) * 128],
                                 ones_r[:], start=False, stop=True)

            w_z = wpool.tile([128, KT, HD], f16, tag="w_z", name="w_z")
            nc.sync.dma_start(w_z[:], i_wz[l])
            z_ps = pp.tile([128, NBLK, T], f32, tag="mid", name="z_ps")
            for mt in range(NBLK):
                for kt in range(KT):
                    nc.tensor.matmul(z_ps[:, mt, :],
                                     w_z[:, kt, mt * 128:(mt + 1) * 128],
                                     xhat[:, kt, :],
                                     start=(kt == 0), stop=(kt == KT - 1))
            zc = apool.tile([128, NBLK, T], f16, tag="zc", name="zc")
            nc.scalar.copy(zc[:], z_ps[:])

            # ---- u = silu(xc)  (sigmoid via exp/ln: one ACT table set)
            e1 = apool.tile([128, FBLK, T], f16, tag="e1", name="e1")
            nc.scalar.activation(e1[:], xc_ps[:], AFT.Exp, scale=-1.0)
            sp1 = apool.tile([128, FBLK, T], f16, tag="sp1", name="sp1")
            nc.scalar.activation(sp1[:], e1[:], AFT.Ln, bias=1.0)
            sg = apool.tile([128, FBLK, T], f16, tag="sg", name="sg")
            nc.scalar.activation(sg[:], sp1[:], AFT.Exp, scale=-1.0)
            u = apool.tile([128, FBLK, T], f16, tag="u", name="u")
            nc.vector.tensor_mul(u[:], xc_ps[:], sg[:])

            # ---- m1 = silu(z) gate
            e1z = apool.tile([128, NBLK, T], f16, tag="e1z", name="e1z")
            nc.scalar.activation(e1z[:], zc[:], AFT.Exp, scale=-1.0)
            spz = apool.tile([128, NBLK, T], f16, tag="spz", name="spz")
            nc.scalar.activation(spz[:], e1z[:], AFT.Ln, bias=1.0)
            sgz = apool.tile([128, NBLK, T], f16, tag="sgz", name="sgz")
            nc.scalar.activation(sgz[:], spz[:], AFT.Exp, scale=-1.0)
            m1 = apool.tile([128, NBLK, T], f16, tag="m1", name="m1")
            nc.vector.tensor_mul(m1[:], zc[:], sgz[:])

            # ---- x_proj over full DI (replicated on both half-cores)
            w_xp = wpool.tile([128, FBLK, RN], f16, tag="w_xp", name="w_xp")
            nc.sync.dma_start(w_xp[:], i_wxp[l])
            dbc_ps = pp.tile([RN, T], f32, tag="mid", name="dbc_ps")
            for kt in range(FBLK):
                nc.tensor.matmul(dbc_ps[:], w_xp[:, kt, :], u[:, kt, :],
                                 start=(kt == 0), stop=(kt == FBLK - 1))
            dbc_s = apool.tile([RN, T], f16, tag="dbc_s", name="dbc_s")
            nc.scalar.copy(dbc_s[:], dbc_ps[:])

            # ---- replicate B,C rows to all partitions via DRAM bounce
            bc_dram = dpool.tile([2 * N, T], f16, tag="bc_dram", name="bc_dram")
            nc.sync.dma_start(bc_dram[:], dbc_s[R:RN, :])
            bcrep = apool.tile([128, 2, N, T], f16, tag="bcrep", name="bcrep")
            nc.sync.dma_start(
                bcrep[:],
                bc_dram[:].rearrange("a t -> (a t)").unsqueeze(0)
                .broadcast_to([128, 2 * N * T])
                .rearrange("p (c n t) -> p c n t", c=2, n=N))

            # ---- delta = softplus(dt @ dtp_w.T + dtp_b)
            w_dt = wpool.tile([32, HD], f16, tag="w_dt", name="w_dt")
            nc.sync.dma_start(w_dt[:], i_wdt[l])
            dp_ps = pp.tile([128, NBLK, T], f32, tag="mid", name="dp_ps")
            for mt in range(NBLK):
                nc.tensor.matmul(dp_ps[:, mt, :],
                                 w_dt[:, mt * 128:(mt + 1) * 128],
                                 dbc_s[0:R, :], start=True, stop=True)
            dex = apool.tile([128, NBLK, T], f16, tag="dex", name="dex")
            for mt in range(NBLK):
                nc.scalar.activation(dex[:, mt, :], dp_ps[:, mt, :], AFT.Exp,
                                     bias=dtb[:, l, mt:mt + 1])
            delta = apool.tile([128, NBLK, T], f16, tag="delta", name="delta")
            nc.scalar.activation(delta[:], dex[:], AFT.Ln, bias=1.0)

            du = apool.tile([128, NBLK, T], f16, tag="du", name="du")
            nc.vector.tensor_mul(du[:], delta[:], u[:, 0:NBLK, :])

            yg = apool.tile([128, NBLK, T], f16, tag="yg", name="yg")

            # ---- selective scan, one 128-channel block at a time
            for d in range(NBLK):
                dA = spool.tile([128, N, T], f16, tag="dA", name="dA")
                n_act = 8 if dve_powers else N
                for n in range(n_act):
                    nc.scalar.activation(dA[:, n, :], delta[:, d, :], AFT.Exp,
                                         scale=asc[:, l, d, n:n + 1])
                if dve_powers:
                    for n in range(8, N):
                        a, b = POW_PAIRS[n]
                        nc.vector.tensor_mul(dA[:, n, :], dA[:, a, :],
                                             dA[:, b, :])
                dBu = s1pool.tile([128, N, T], f16, tag="dBu", name="dBu")
                nc.vector.tensor_mul(
                    dBu[:], du[:, d:d + 1, :].broadcast_to([128, N, T]),
                    bcrep[:, 0])
                h = spool.tile([128, N, T], f16, tag="h", name="h")
                for n in range(N):
                    nc.vector.tensor_tensor_scan(
                        h[:, n, :], dA[:, n, :], dBu[:, n, :], 0.0,
                        op0=AOP.mult, op1=AOP.add)
                G = s1pool.tile([128, N, T], f16, tag="G", name="G")
                nc.vector.tensor_mul(G[:], h[:], bcrep[:, 1])
                t8 = s1pool.tile([128, 8, T], f16, tag="t8", name="t8")
                nc.vector.tensor_add(t8[:], G[:, 0:8, :], G[:, 8:16, :])
                t4 = s1pool.tile([128, 4, T], f16, tag="t4", name="t4")
                nc.vector.tensor_add(t4[:], t8[:, 0:4, :], t8[:, 4:8, :])
                t2 = s1pool.tile([128, 2, T], f16, tag="t2", name="t2")
                nc.vector.tensor_add(t2[:], t4[:, 0:2, :], t4[:, 2:4, :])
                y_d = s1pool.tile([128, T], f32, tag="y_d", name="y_d")
                nc.vector.tensor_add(y_d[:], t2[:, 0, :], t2[:, 1, :])
                yd2 = s1pool.tile([128, T], f16, tag="yd2", name="yd2")
                nc.vector.scalar_tensor_tensor(
                    yd2[:], u[:, d, :], dpr[:, l, d:d + 1], y_d[:],
                    op0=AOP.mult, op1=AOP.add)
                nc.vector.tensor_mul(yg[:, d, :], yd2[:], m1[:, d, :])

            # ---- pairwise AllGather of gated halves
            yg_in = dpool.tile([NBLK, 128, T], f16, tag="yg_in", name="yg_in")
            yg_out = dpool.tile([2, NBLK, 128, T], f16, tag="yg_out",
                                name="yg_out")
            nc.sync.dma_start(yg_in[:].rearrange("n p t -> p n t"), yg[:])
            nc.gpsimd.collective_compute(
                "AllGather", AOP.bypass, ins=[yg_in[:].opt()],
                outs=[yg_out[:].opt()], replica_groups=groups)
            ygf = apool.tile([128, FBLK, T], f16, tag="ygf", name="ygf")
            nc.sync.dma_start(
                ygf[:], yg_out[:].rearrange("g n p t -> p (g n) t"))

            # ---- out_proj (K rows in original order) + residual
            w_out = wpool.tile([128, FBLK, D], f16, tag="w_out", name="w_out")
            nc.sync.dma_start(w_out[:], i_wout[l])
            op_ps = pp.tile([128, PBLK, T], f32, tag="big", name="op_ps")
            for mt in range(PBLK):
                for kt in range(FBLK):
                    nc.tensor.matmul(op_ps[:, mt, :],
                                     w_out[:, kt, mt * 128:(mt + 1) * 128],
                                     ygf[:, kt, :],
                                     start=(kt == 0), stop=(kt == FBLK - 1))
            for pb in range(PBLK):
                nc.vector.tensor_add(xTh[:, pb, :], xTh[:, pb, :],
                                     op_ps[:, pb, :])

        # ================================================= final norm + heads
        xf = apool.tile([128, PBLK, T], f16, tag="xhat", name="xf")
        layer_norm(NL, xf)
        w_ps = cpool.tile([128, PBLK, SD], f16, name="w_ps")
        w_pa = cpool.tile([128, PBLK, AD], f16, name="w_pa")
        b_ps = cpool.tile([1, SD], f16, name="b_ps")
        b_pa = cpool.tile([1, AD], f16, name="b_pa")
        nc.sync.dma_start(w_ps[:], i_wps[:])
        nc.sync.dma_start(w_pa[:], i_wpa[:])
        nc.sync.dma_start(b_ps[:], i_bps[:])
        nc.sync.dma_start(b_pa[:], i_bpa[:])

        xf_v = xf[:].rearrange("p b (l k) -> p b l k", k=4)
        sp_ps = pp.tile([SD, L], f32, tag="s1", name="sp_ps")
        for kt in range(PBLK):
            nc.tensor.matmul(sp_ps[:], w_ps[:, kt, :], xf_v[:, kt, :, 3],
                             start=(kt == 0), stop=False)
        nc.tensor.matmul(sp_ps[:], b_ps[:], ones_r[:, 0:L],
                         start=False, stop=True)
        sp_s = apool.tile([SD, L], f32, tag="sp_s", name="sp_s")
        nc.scalar.copy(sp_s[:], sp_ps[:])
        nc.sync.dma_start(o_sp[:], sp_s[:])

        ap_ps = pp.tile([AD, L], f32, tag="s2", name="ap_ps")
        for kt in range(PBLK):
            nc.tensor.matmul(ap_ps[:], w_pa[:, kt, :], xf_v[:, kt, :, 2],
                             start=(kt == 0), stop=False)
        nc.tensor.matmul(ap_ps[:], b_pa[:], ones_r[:, 0:L],
                         start=False, stop=True)
        ap_s = apool.tile([AD, L], f32, tag="ap_s", name="ap_s")
        nc.scalar.copy(ap_s[:], ap_ps[:])
        nc.sync.dma_start(o_ap[:], ap_s[:])

    nc.finalize()
    return nc


# =============================================================== marshaling
def _marshal_core(inputs, b, half):
    f = np.float16
    perm = (np.r_[HD:DI, 0:HD] if half == 1 else np.r_[0:DI])   # own half first

    in_w = inputs["in_w"]          # (NL, 2*DI, D)
    conv_w = inputs["conv_w"]      # (NL, DI, DC)
    inw_m = in_w[:, :DI, :][:, perm, :]                          # (NL, DI, D)
    conv_p = conv_w[:, perm, :]
    inw_r = np.ascontiguousarray(inw_m.transpose(0, 2, 1)).reshape(NL, KT, 128, DI)
    convT = conv_p.transpose(0, 2, 1)                            # (NL, DC, DI)
    w_in = (inw_r[:, None] * convT[:, :, None, None, :]).transpose(0, 1, 3, 2, 4)
    w_in = np.ascontiguousarray(w_in, dtype=f)     # (NL, DC, 128, KT, DI)

    zrows = in_w[:, DI + half * HD: DI + (half + 1) * HD, :]     # (NL, HD, D)
    w_z = np.ascontiguousarray(
        zrows.transpose(0, 2, 1).reshape(NL, KT, 128, HD).transpose(0, 2, 1, 3),
        dtype=f)

    xp_w = inputs["xp_w"][:, :, perm]                            # (NL, RN, DI)
    w_xp = np.ascontiguousarray(
        xp_w.transpose(0, 2, 1).reshape(NL, FBLK, 128, RN).transpose(0, 2, 1, 3),
        dtype=f)

    own = slice(half * HD, (half + 1) * HD)
    w_dt = np.ascontiguousarray(
        inputs["dtp_w"][:, own, :].transpose(0, 2, 1), dtype=f)  # (NL, 32, HD)
    dtb = np.ascontiguousarray(
        inputs["dtp_b"][:, own].reshape(NL, NBLK, 128).transpose(2, 0, 1),
        dtype=np.float32)                                        # (128, NL, NBLK)
    A = -np.exp(inputs["A_log"].astype(np.float64)).astype(np.float32)
    asc = np.ascontiguousarray(
        A[:, own, :].reshape(NL, NBLK, 128, N).transpose(2, 0, 1, 3),
        dtype=np.float32)                                        # (128, NL, NBLK, N)
    dpr = np.ascontiguousarray(
        inputs["Dp"][:, own].reshape(NL, NBLK, 128).transpose(2, 0, 1),
        dtype=np.float32)
    b_cv = np.ascontiguousarray(
        inputs["conv_b"][:, perm].reshape(NL, 1, DI), dtype=f)

    out_w = inputs["out_w"]                                      # (NL, D, DI)
    w_out = np.ascontiguousarray(
        out_w.transpose(0, 2, 1).reshape(NL, FBLK, 128, D).transpose(0, 2, 1, 3),
        dtype=f)

    ln_w = np.concatenate([inputs["ln_w"], inputs["fn_w"][None]], 0)
    ln_b = np.concatenate([inputs["ln_b"], inputs["fn_b"][None]], 0)
    lnw = np.ascontiguousarray(
        ln_w.reshape(NL + 1, PBLK, 128).transpose(2, 0, 1), dtype=np.float32)
    lnb = np.ascontiguousarray(
        ln_b.reshape(NL + 1, PBLK, 128).transpose(2, 0, 1), dtype=np.float32)

    te = inputs["et_w"][inputs["timesteps"][b]]                  # (L, D)
    teT = np.ascontiguousarray(
        te.T.reshape(PBLK, 128, L).transpose(1, 0, 2), dtype=f)

    w_psh = np.ascontiguousarray(
        inputs["ps_w"].T.reshape(PBLK, 128, SD).transpose(1, 0, 2), dtype=f)
    w_pah = np.ascontiguousarray(
        inputs["pa_w"].T.reshape(PBLK, 128, AD).transpose(1, 0, 2), dtype=f)

    return {
        "i_sT": np.ascontiguousarray(inputs["states"][b].T, dtype=f),
        "i_aT": np.ascontiguousarray(inputs["actions"][b].T, dtype=f),
        "i_rcT": np.ascontiguousarray(
            np.concatenate([inputs["returns_to_go"][b].T,
                            inputs["constraints_to_go"][b].T], 1), dtype=f),
        "i_teT": teT,
        "i_wes": np.ascontiguousarray(inputs["es_w"].T, dtype=f),
        "i_wea": np.ascontiguousarray(inputs["ea_w"].T, dtype=f),
        "i_werc": np.ascontiguousarray(
            np.concatenate([inputs["er_w"].T, inputs["ec_w"].T], 1), dtype=f),
        "i_bemb": np.ascontiguousarray(
            np.concatenate([inputs["es_b"], inputs["ea_b"],
                            inputs["er_b"], inputs["ec_b"]])[None], dtype=f),
        "i_ones": np.ones((1, T), dtype=f),
        "i_onesp": np.ones((1, 128), dtype=f),
        "i_wstat": np.full((128, 1), 1.0 / D, dtype=f),
        "i_lnw": lnw, "i_lnb": lnb,
        "i_asc": asc, "i_dpr": dpr, "i_dtb": dtb,
        "i_wps": w_psh, "i_wpa": w_pah,
        "i_bps": np.ascontiguousarray(inputs["ps_b"][None], dtype=f),
        "i_bpa": np.ascontiguousarray(inputs["pa_b"][None], dtype=f),
        "i_win": w_in, "i_wz": w_z, "i_wxp": w_xp, "i_wdt": w_dt,
        "i_bcv": b_cv, "i_wout": w_out,
    }


def _check_dve_powers(inputs) -> bool:
    c = np.exp(inputs["A_log"].astype(np.float64))
    for n, (a, b) in POW_PAIRS.items():
        if not np.allclose(c[..., n], c[..., a] + c[..., b], rtol=1e-4):
            return False
    return True


def marshal_inputs(inputs, n_pairs=B):
    inputs = {k: np.asarray(v) for k, v in inputs.items()}
    dve_ok = _check_dve_powers(inputs)
    in_maps = []
    for b in range(n_pairs):
        for half in (0, 1):
            in_maps.append(_marshal_core(inputs, b, half))
    return in_maps, dve_ok


# =============================================================== entry point
_CACHE = {}


def get_program(n_pairs, dve_ok=True):
    key = (n_pairs, dve_ok)
    if key not in _CACHE:
        _CACHE[key] = build_program(n_pairs, dve_powers=dve_ok)
    return _CACHE[key]


def kernel(**inputs):
    in_maps, dve_ok = marshal_inputs(inputs, n_pairs=B)
    nc = get_program(B, dve_ok)
    res = run_bass_kernel_spmd(nc, in_maps, list(range(2 * B)))
    sp = np.stack([res.results[2 * b]["o_spT"].T for b in range(B)], 0)
    ap = np.stack([res.results[2 * b]["o_apT"].T for b in range(B)], 0)
    return sp.astype(np.float32), ap.astype(np.float32)


# revision 18
# speedup vs baseline: 1.0932x; 1.0932x over previous
"""Trainium2 Bass kernel for nn_DeepMambaModel (decision-transformer-style Mamba).

Sharding: 8 cores = 4 batch groups x 2 d_inner halves (Megatron-style TP on the
Mamba inner dim).  Each pair of cores (2b, 2b+1) handles batch b; within the
pair the selective-scan region (the elementwise-heavy (T, DI, N) work) is split
over d_inner halves, the small matmuls (in_proj/conv/x_proj) are replicated,
and the only cross-core traffic is one pairwise AllGather of the gated scan
output per layer.

Layout: activations live transposed on-chip as [feature-partition, token-free]
tiles so every matmul is lhsT.T @ rhs with K on partitions.  The causal
depthwise conv is folded into in_proj as 4 shifted accumulating matmuls.
dA = exp(A[n] * delta) is produced by ScalarE exp-with-scale for n=0..7 and by
exact exponent-addition products on VectorE for n=8..15 (c_n = n+1 here).  The
scan itself is tensor_tensor_scan (state = dA*state + dBu) along tokens.
All transcendentals (softplus, silu, rsqrt) are built from exp/ln so a single
ACT table set is used for the whole kernel.

Per-core channel permutation: each core's d_inner channels are ordered
[own half, other half] in its shipped weights, so the own half is always block
0..NBLK-1 of u/xc with an identical program on every core; the AllGather output
is in original order (rank0 half first), so out_w ships unpermuted.
"""

import numpy as np

import concourse.bass as bass
import concourse.bacc as bacc
import concourse.mybir as mybir
import concourse.tile as tile
from concourse.bass_utils import run_bass_kernel_spmd

# ---------------------------------------------------------------- dimensions
B, L, D = 4, 64, 512
DI, N, DC, R = 1024, 16, 4, 32
NL = 4
SD, AD, MEL = 17, 6, 1000
EPS = 1e-5

T = 4 * L                  # 256 interleaved tokens
HD = DI // 2               # d_inner half per core
KT = D // 128              # 4   k-tiles over d_model
PBLK = D // 128            # 4   partition blocks of the residual stream
NBLK = HD // 128           # 4   partition blocks of the owned d_inner half
FBLK = DI // 128           # 8   partition blocks of full d_inner
RN = R + 2 * N             # 64  x_proj output channels

f16 = mybir.dt.float16
f32 = mybir.dt.float32
AOP = mybir.AluOpType
AFT = mybir.ActivationFunctionType

# exponent-addition pairs: dA[n] = dA[a] * dA[b]  (valid when c_n ~ n+1)
POW_PAIRS = {12: (5, 6), 13: (6, 6), 14: (6, 7), 15: (7, 7)}
N_ACT_POW = 12


def _patch_act_tables():
    """Force every activation onto the natural_log_exp table set (covers
    exp/ln/square/copy) so the kernel never reloads ACT tables mid-run."""
    from concourse import hw_specs
    if getattr(bacc, "_act_tables_patched", False):
        return
    orig = hw_specs.get_activation_tables

    def only_lnexp(arch):
        t = orig(arch)
        return {k: (v if k == "natural_log_exp_and_others" else set())
                for k, v in t.items()}

    bacc.get_activation_tables = only_lnexp
    bacc._act_tables_patched = True


# =============================================================== program
def build_program(n_pairs: int, dve_powers: bool = True) -> bass.Bass:
    _patch_act_tables()
    nc = bacc.Bacc()
    # register the extra float constant used as an activation bias (eps)
    _ct = nc.alloc_sbuf_tensor(f"const-f32-eps", [128, 1], f32)
    nc.gpsimd.memset(_ct.ap(), EPS)
    nc.const_aps.aps[(f32, EPS)] = _ct.ap()
    nc.all_engine_barrier()
    dp = nc.declare_dram_parameter

    i_sT = dp("i_sT", [SD, L], f16, isOutput=False)
    i_aT = dp("i_aT", [AD, L], f16, isOutput=False)
    i_rcT = dp("i_rcT", [1, 2 * L], f16, isOutput=False)
    i_teT = dp("i_teT", [128, PBLK, L], f16, isOutput=False)
    i_wes = dp("i_wes", [SD, D], f16, isOutput=False)
    i_wea = dp("i_wea", [AD, D], f16, isOutput=False)
    i_werc = dp("i_werc", [1, 2 * D], f16, isOutput=False)
    i_bemb = dp("i_bemb", [1, 4 * D], f16, isOutput=False)
    i_ones = dp("i_ones", [1, T], f16, isOutput=False)
    i_onesp = dp("i_onesp", [1, 128], f16, isOutput=False)
    i_wstat = dp("i_wstat", [128, 1], f16, isOutput=False)
    i_lnw = dp("i_lnw", [128, NL + 1, PBLK], f32, isOutput=False)
    i_lnb = dp("i_lnb", [128, NL + 1, PBLK], f32, isOutput=False)
    i_asc = dp("i_asc", [128, NL, NBLK, N], f32, isOutput=False)
    i_dpr = dp("i_dpr", [128, NL, NBLK], f32, isOutput=False)
    i_dtb = dp("i_dtb", [128, NL, NBLK], f32, isOutput=False)
    i_wps = dp("i_wps", [128, PBLK, SD], f16, isOutput=False)
    i_wpa = dp("i_wpa", [128, PBLK, AD], f16, isOutput=False)
    i_bps = dp("i_bps", [1, SD], f16, isOutput=False)
    i_bpa = dp("i_bpa", [1, AD], f16, isOutput=False)
    i_win = dp("i_win", [NL, 128, KT, DI], f16, isOutput=False)
    i_wcv = dp("i_wcv", [NL, 128, FBLK, DC], f32, isOutput=False)
    i_wz = dp("i_wz", [NL, 128, KT, HD], f16, isOutput=False)
    i_wxp = dp("i_wxp", [NL, 128, FBLK, RN], f16, isOutput=False)
    i_wdt = dp("i_wdt", [NL, 32, HD], f16, isOutput=False)
    i_bcv = dp("i_bcv", [NL, 128, FBLK], f16, isOutput=False)
    i_wout = dp("i_wout", [NL, 128, FBLK, D], f16, isOutput=False)

    o_sp = dp("o_spT", [SD, L], f32, isOutput=True)
    o_ap = dp("o_apT", [AD, L], f32, isOutput=True)

    groups = [[2 * i, 2 * i + 1] for i in range(n_pairs)]

    from contextlib import ExitStack

    with tile.TileContext(nc) as tc, ExitStack() as es:
        cpool = es.enter_context(tc.tile_pool(name="consts", bufs=1))
        wpool = es.enter_context(tc.tile_pool(name="weights", bufs=2))
        apool = es.enter_context(tc.tile_pool(name="acts", bufs=1))
        spool = es.enter_context(tc.tile_pool(name="scan", bufs=1))
        s1pool = es.enter_context(tc.tile_pool(name="scan1", bufs=1))
        pp = es.enter_context(tc.tile_pool(name="ps", bufs=1, space="PSUM"))
        dpool = es.enter_context(tc.tile_pool(name="drampool", bufs=1, space="DRAM"))

        # ---- persistent tiles / constants
        xTh = cpool.tile([128, PBLK, T], f16, name="xTh")     # residual stream
        ones_r = cpool.tile([1, T], f16, name="ones_r")
        ones_p = cpool.tile([1, 128], f16, name="ones_p")
        wstat = cpool.tile([128, 1], f16, name="wstat")
        lnw = cpool.tile([128, NL + 1, PBLK], f32, name="lnw")
        lnb = cpool.tile([128, NL + 1, PBLK], f32, name="lnb")
        asc = cpool.tile([128, NL, NBLK, N], f32, name="asc")
        dpr = cpool.tile([128, NL, NBLK], f32, name="dpr")
        dtb = cpool.tile([128, NL, NBLK], f32, name="dtb")
        teT = cpool.tile([128, PBLK, L], f16, name="teT")

        nc.sync.dma_start(ones_r[:], i_ones[:])
        nc.sync.dma_start(ones_p[:], i_onesp[:])
        nc.sync.dma_start(wstat[:], i_wstat[:])
        nc.sync.dma_start(lnw[:], i_lnw[:])
        nc.sync.dma_start(lnb[:], i_lnb[:])
        nc.sync.dma_start(asc[:], i_asc[:])
        nc.sync.dma_start(dpr[:], i_dpr[:])
        nc.sync.dma_start(dtb[:], i_dtb[:])
        nc.sync.dma_start(teT[:], i_teT[:])

        # ================================================= embeddings
        w_es = cpool.tile([SD, D], f16, name="w_es")
        w_ea = cpool.tile([AD, D], f16, name="w_ea")
        w_erc = cpool.tile([1, 2 * D], f16, name="w_erc")
        b_emb = cpool.tile([1, 4 * D], f16, name="b_emb")
        sT = cpool.tile([SD, L], f16, name="sT")
        aT = cpool.tile([AD, L], f16, name="aT")
        rcT = cpool.tile([1, 2 * L], f16, name="rcT")
        nc.sync.dma_start(w_es[:], i_wes[:])
        nc.sync.dma_start(w_ea[:], i_wea[:])
        nc.sync.dma_start(w_erc[:], i_werc[:])
        nc.sync.dma_start(b_emb[:], i_bemb[:])
        nc.sync.dma_start(sT[:], i_sT[:])
        nc.sync.dma_start(aT[:], i_aT[:])
        nc.sync.dma_start(rcT[:], i_rcT[:])

        # token order k = 0:return 1:constraint 2:state 3:action
        xTh_v = xTh[:].rearrange("p b (l k) -> p b l k", k=4)
        for pb in range(PBLK):
            csl = slice(pb * 128, (pb + 1) * 128)
            streams = [
                (w_erc[:, pb * 128:pb * 128 + 128], rcT[:, 0:L], 2),
                (w_erc[:, D + pb * 128:D + pb * 128 + 128], rcT[:, L:2 * L], 3),
                (w_es[:, csl], sT[:], 0),
                (w_ea[:, csl], aT[:], 1),
            ]
            for k, (wT, rhs, brow) in enumerate(streams):
                e_ps = pp.tile([128, L], f32, tag=("s1" if k % 2 == 0 else "s2"),
                               name="e_ps")
                nc.tensor.matmul(e_ps[:], wT, rhs, start=True, stop=False)
                nc.tensor.matmul(
                    e_ps[:], b_emb[:, brow * D + pb * 128:brow * D + pb * 128 + 128],
                    ones_r[:, 0:L], start=False, stop=True)
                nc.vector.tensor_add(xTh_v[:, pb, :, k], e_ps[:], teT[:, pb, :])

        # ================================================= layernorm helper
        def layer_norm(l_idx, out_xhat):
            sq = apool.tile([128, PBLK, T], f16, tag="ln_sq", name="sq")
            nc.scalar.activation(sq[:], xTh[:], AFT.Square)
            mu_ps = pp.tile([1, T], f32, tag="s1", name="mu_ps")
            ex_ps = pp.tile([1, T], f32, tag="s2", name="ex_ps")
            for pb in range(PBLK):
                nc.tensor.matmul(mu_ps[:], wstat[:], xTh[:, pb, :],
                                 start=(pb == 0), stop=(pb == PBLK - 1))
            for pb in range(PBLK):
                nc.tensor.matmul(ex_ps[:], wstat[:], sq[:, pb, :],
                                 start=(pb == 0), stop=(pb == PBLK - 1))
            musq = apool.tile([1, T], f32, tag="ln_s1", name="musq")
            nc.scalar.activation(musq[:], mu_ps[:], AFT.Square)
            mu_s = apool.tile([1, T], f16, tag="ln_s2", name="mu_s")
            nc.scalar.copy(mu_s[:], mu_ps[:])
            var_s = apool.tile([1, T], f32, tag="ln_s3", name="var_s")
            nc.vector.tensor_tensor(var_s[:], ex_ps[:], musq[:], AOP.subtract)
            lv = apool.tile([1, T], f32, tag="ln_s4", name="lv")
            nc.scalar.activation(lv[:], var_s[:], AFT.Ln, bias=EPS)
            rstd = apool.tile([1, T], f16, tag="ln_s5", name="rstd")
            nc.scalar.activation(rstd[:], lv[:], AFT.Exp, scale=-0.5)
            mur_ps = pp.tile([128, T], f32, tag="s1", name="mur_ps")
            rsr_ps = pp.tile([128, T], f32, tag="s2", name="rsr_ps")
            nc.tensor.matmul(mur_ps[:], ones_p[:], mu_s[:], start=True, stop=True)
            nc.tensor.matmul(rsr_ps[:], ones_p[:], rstd[:], start=True, stop=True)
            for pb in range(PBLK):
                cen = apool.tile([128, T], f32, tag="ln_cen", name="cen")
                nc.vector.tensor_tensor(cen[:], xTh[:, pb, :], mur_ps[:],
                                        AOP.subtract)
                pre = apool.tile([128, T], f32, tag="ln_pre", name="pre")
                nc.vector.tensor_tensor(pre[:], cen[:], rsr_ps[:], AOP.mult)
                nc.vector.scalar_tensor_tensor(
                    out_xhat[:, pb, :], pre[:], lnw[:, l_idx, pb:pb + 1],
                    lnb[:, l_idx, pb:pb + 1].broadcast_to([128, T]),
                    op0=AOP.mult, op1=AOP.add)

        # ================================================= layers
        for l in range(NL):
            xhat = apool.tile([128, PBLK, T], f16, tag="xhat", name="xhat")
            layer_norm(l, xhat)

            # ---- in_proj (xm branch) -> psum -> sbuf, then depthwise conv
            xm_ps = pp.tile([128, FBLK, T], f32, tag="big", name="xm_ps")
            w_in = wpool.tile([128, KT, DI], f16, tag="w_in", name="w_in")
            nc.sync.dma_start(w_in[:], i_win[l])
            wcv = wpool.tile([128, FBLK, DC], f32, tag="wcv", name="wcv")
            nc.sync.dma_start(wcv[:], i_wcv[l])
            bcv = wpool.tile([128, FBLK], f16, tag="bcv", name="bcv")
            nc.sync.dma_start(bcv[:], i_bcv[l])
            for mt in range(FBLK):
                for kt in range(KT):
                    nc.tensor.matmul(xm_ps[:, mt, :],
                                     w_in[:, kt, mt * 128:(mt + 1) * 128],
                                     xhat[:, kt, :],
                                     start=(kt == 0), stop=(kt == KT - 1))
            # copy to sbuf with a 3-column zero pad in front of each block
            xm_s = apool.tile([128, FBLK, T + 3], f16, tag="xm_s", name="xm_s")
            nc.vector.memset(xm_s[:, :, 0:3], 0.0)
            nc.scalar.copy(xm_s[:, :, 3:T + 3], xm_ps[:])
            # causal depthwise conv: xc[t] = sum_k w_k * xm[t-3+k] + b
            # (conv bias rides the first stt's in1 as a broadcast plane)
            xc_s = apool.tile([128, FBLK, T], f16, tag="xc_s", name="xc_s")
            for mt in range(FBLK):
                eng = nc.vector
                for k in range(DC):
                    in1 = (bcv[:, mt:mt + 1].broadcast_to([128, T]) if k == 0
                           else xc_s[:, mt, :])
                    eng.scalar_tensor_tensor(
                        xc_s[:, mt, :], xm_s[:, mt, k:k + T],
                        wcv[:, mt, k:k + 1], in1, op0=AOP.mult, op1=AOP.add)

            w_z = wpool.tile([128, KT, HD], f16, tag="w_z", name="w_z")
            nc.sync.dma_start(w_z[:], i_wz[l])
            z_ps = pp.tile([128, NBLK, T], f32, tag="mid", name="z_ps")
            for mt in range(NBLK):
                for kt in range(KT):
                    nc.tensor.matmul(z_ps[:, mt, :],
                                     w_z[:, kt, mt * 128:(mt + 1) * 128],
                                     xhat[:, kt, :],
                                     start=(kt == 0), stop=(kt == KT - 1))
            zc = apool.tile([128, NBLK, T], f16, tag="zc", name="zc")
            nc.scalar.copy(zc[:], z_ps[:])

            # ---- u = silu(xc)  (sigmoid via exp/ln: one ACT table set)
            sg = apool.tile([128, FBLK, T], f16, tag="sg", name="sg")
            nc.scalar.activation(sg[:], xc_s[:], AFT.Exp, scale=-1.0)
            nc.scalar.activation(sg[:], sg[:], AFT.Ln, bias=1.0)
            nc.scalar.activation(sg[:], sg[:], AFT.Exp, scale=-1.0)
            u = apool.tile([128, FBLK, T], f16, tag="u", name="u")
            nc.vector.tensor_mul(u[:], xc_s[:], sg[:])

            # ---- m1 = silu(z) gate
            sgz = apool.tile([128, NBLK, T], f16, tag="sgz", name="sgz")
            nc.scalar.activation(sgz[:], zc[:], AFT.Exp, scale=-1.0)
            nc.scalar.activation(sgz[:], sgz[:], AFT.Ln, bias=1.0)
            nc.scalar.activation(sgz[:], sgz[:], AFT.Exp, scale=-1.0)
            m1 = apool.tile([128, NBLK, T], f16, tag="m1", name="m1")
            nc.vector.tensor_mul(m1[:], zc[:], sgz[:])

            # ---- x_proj over full DI (replicated on both half-cores)
            w_xp = wpool.tile([128, FBLK, RN], f16, tag="w_xp", name="w_xp")
            nc.sync.dma_start(w_xp[:], i_wxp[l])
            dbc_ps = pp.tile([RN, T], f32, tag="mid", name="dbc_ps")
            for kt in range(FBLK):
                nc.tensor.matmul(dbc_ps[:], w_xp[:, kt, :], u[:, kt, :],
                                 start=(kt == 0), stop=(kt == FBLK - 1))
            dbc_s = apool.tile([RN, T], f16, tag="dbc_s", name="dbc_s")
            nc.scalar.copy(dbc_s[:], dbc_ps[:])

            # ---- replicate B,C rows to all partitions via DRAM bounce
            bc_dram = dpool.tile([2 * N, T], f16, tag="bc_dram", name="bc_dram")
            nc.sync.dma_start(bc_dram[:], dbc_s[R:RN, :])
            bcrep = apool.tile([128, 2, N, T], f16, tag="bcrep", name="bcrep")
            nc.sync.dma_start(
                bcrep[:],
                bc_dram[:].rearrange("a t -> (a t)").unsqueeze(0)
                .broadcast_to([128, 2 * N * T])
                .rearrange("p (c n t) -> p c n t", c=2, n=N))

            # ---- delta = softplus(dt @ dtp_w.T + dtp_b)
            w_dt = wpool.tile([32, HD], f16, tag="w_dt", name="w_dt")
            nc.sync.dma_start(w_dt[:], i_wdt[l])
            dp_ps = pp.tile([128, NBLK, T], f32, tag="mid", name="dp_ps")
            for mt in range(NBLK):
                nc.tensor.matmul(dp_ps[:, mt, :],
                                 w_dt[:, mt * 128:(mt + 1) * 128],
                                 dbc_s[0:R, :], start=True, stop=True)
            dex = apool.tile([128, NBLK, T], f16, tag="dex", name="dex")
            for mt in range(NBLK):
                nc.scalar.activation(dex[:, mt, :], dp_ps[:, mt, :], AFT.Exp,
                                     bias=dtb[:, l, mt:mt + 1])
            delta = apool.tile([128, NBLK, T], f16, tag="delta", name="delta")
            nc.scalar.activation(delta[:], dex[:], AFT.Ln, bias=1.0)

            du = apool.tile([128, NBLK, T], f16, tag="du", name="du")
            nc.vector.tensor_mul(du[:], delta[:], u[:, 0:NBLK, :])

            yg = apool.tile([128, NBLK, T], f16, tag="yg", name="yg")

            # ---- selective scan over pairs of 128-channel blocks.
            # All 16 state lanes of one block are chained into a single
            # tensor_tensor_scan: a zero column between lanes multiplies the
            # carried state by 0 and adds 0, resetting it exactly.
            TP = T + 1
            for q in range(NBLK // 2):          # dblk pairs (d0, d0+1)
                d0 = 2 * q
                dA = spool.tile([128, 2, N, TP], f16, tag="dA", name="dA")
                nc.vector.memset(dA[:, :, :, T:TP], 0.0)
                if dve_powers:
                    # A is d-independent: one pair-fused ACT per state lane
                    for n in range(N_ACT_POW):
                        nc.scalar.activation(
                            dA[:, :, n, 0:T],
                            delta[:, d0:d0 + 2, :], AFT.Exp,
                            scale=asc[:, l, d0, n:n + 1])
                    for n in range(N_ACT_POW, N):
                        a, b = POW_PAIRS[n]
                        nc.vector.tensor_mul(dA[:, :, n, 0:T], dA[:, :, a, 0:T],
                                             dA[:, :, b, 0:T])
                else:
                    for i in range(2):
                        for n in range(N):
                            nc.scalar.activation(
                                dA[:, i, n, 0:T],
                                delta[:, d0 + i, :], AFT.Exp,
                                scale=asc[:, l, d0 + i, n:n + 1])
                dBu = s1pool.tile([128, 2, N, TP], f16, tag="dBu", name="dBu")
                nc.vector.memset(dBu[:, :, :, T:TP], 0.0)
                nc.vector.tensor_mul(
                    dBu[:, :, :, 0:T],
                    du[:, d0:d0 + 2, :].unsqueeze(2).broadcast_to([128, 2, N, T]),
                    bcrep[:, 0].unsqueeze(1).broadcast_to([128, 2, N, T]))
                h = spool.tile([128, 2, N, TP], f16, tag="h", name="h")
                for i in range(2):
                    nc.vector.tensor_tensor_scan(
                        h[:, i].rearrange("p n t -> p (n t)"),
                        dA[:, i].rearrange("p n t -> p (n t)"),
                        dBu[:, i].rearrange("p n t -> p (n t)"), 0.0,
                        op0=AOP.mult, op1=AOP.add)
                # G = h * C  (overwrites dBu's slot)
                nc.vector.tensor_mul(
                    dBu[:, :, :, 0:T], h[:, :, :, 0:T],
                    bcrep[:, 1].unsqueeze(1).broadcast_to([128, 2, N, T]))
                G = dBu
                t8 = s1pool.tile([128, 2, 8, T], f16, tag="t8", name="t8")
                nc.vector.tensor_add(t8[:], G[:, :, 0:8, 0:T], G[:, :, 8:16, 0:T])
                t4 = s1pool.tile([128, 2, 4, T], f16, tag="t4", name="t4")
                nc.vector.tensor_add(t4[:], t8[:, :, 0:4, :], t8[:, :, 4:8, :])
                t2 = s1pool.tile([128, 2, 2, T], f16, tag="t2", name="t2")
                nc.vector.tensor_add(t2[:], t4[:, :, 0:2, :], t4[:, :, 2:4, :])
                y_d = s1pool.tile([128, 2, T], f32, tag="y_d", name="y_d")
                nc.vector.tensor_add(y_d[:], t2[:, :, 0, :], t2[:, :, 1, :])
                for i in range(2):
                    d = d0 + i
                    yd2 = s1pool.tile([128, T], f16, tag="yd2", name="yd2")
                    nc.vector.scalar_tensor_tensor(
                        yd2[:], u[:, d, :], dpr[:, l, d:d + 1], y_d[:, i, :],
                        op0=AOP.mult, op1=AOP.add)
                    nc.vector.tensor_mul(yg[:, d, :], yd2[:], m1[:, d, :])

            # ---- pairwise AllGather of gated halves
            yg_in = dpool.tile([NBLK, 128, T], f16, tag="yg_in", name="yg_in")
            yg_out = dpool.tile([2, NBLK, 128, T], f16, tag="yg_out",
                                name="yg_out")
            nc.sync.dma_start(yg_in[:].rearrange("n p t -> p n t"), yg[:])
            nc.gpsimd.collective_compute(
                "AllGather", AOP.bypass, ins=[yg_in[:].opt()],
                outs=[yg_out[:].opt()], replica_groups=groups)
            ygf = apool.tile([128, FBLK, T], f16, tag="ygf", name="ygf")
            nc.sync.dma_start(
                ygf[:], yg_out[:].rearrange("g n p t -> p (g n) t"))

            # ---- out_proj (K rows in original order) + residual
            w_out = wpool.tile([128, FBLK, D], f16, tag="w_out", name="w_out")
            nc.sync.dma_start(w_out[:], i_wout[l])
            op_ps = pp.tile([128, PBLK, T], f32, tag="big", name="op_ps")
            for mt in range(PBLK):
                for kt in range(FBLK):
                    nc.tensor.matmul(op_ps[:, mt, :],
                                     w_out[:, kt, mt * 128:(mt + 1) * 128],
                                     ygf[:, kt, :],
                                     start=(kt == 0), stop=(kt == FBLK - 1))
            for pb in range(PBLK):
                nc.vector.tensor_add(xTh[:, pb, :], xTh[:, pb, :],
                                     op_ps[:, pb, :])

        # ================================================= final norm + heads
        xf = apool.tile([128, PBLK, T], f16, tag="xhat", name="xf")
        layer_norm(NL, xf)
        w_ps = cpool.tile([128, PBLK, SD], f16, name="w_ps")
        w_pa = cpool.tile([128, PBLK, AD], f16, name="w_pa")
        b_ps = cpool.tile([1, SD], f16, name="b_ps")
        b_pa = cpool.tile([1, AD], f16, name="b_pa")
        nc.sync.dma_start(w_ps[:], i_wps[:])
        nc.sync.dma_start(w_pa[:], i_wpa[:])
        nc.sync.dma_start(b_ps[:], i_bps[:])
        nc.sync.dma_start(b_pa[:], i_bpa[:])

        xf_v = xf[:].rearrange("p b (l k) -> p b l k", k=4)
        sp_ps = pp.tile([SD, L], f32, tag="s1", name="sp_ps")
        for kt in range(PBLK):
            nc.tensor.matmul(sp_ps[:], w_ps[:, kt, :], xf_v[:, kt, :, 3],
                             start=(kt == 0), stop=False)
        nc.tensor.matmul(sp_ps[:], b_ps[:], ones_r[:, 0:L],
                         start=False, stop=True)
        sp_s = apool.tile([SD, L], f32, tag="sp_s", name="sp_s")
        nc.scalar.copy(sp_s[:], sp_ps[:])
        nc.sync.dma_start(o_sp[:], sp_s[:])

        ap_ps = pp.tile([AD, L], f32, tag="s2", name="ap_ps")
        for kt in range(PBLK):
            nc.tensor.matmul(ap_ps[:], w_pa[:, kt, :], xf_v[:, kt, :, 2],
                             start=(kt == 0), stop=False)
        nc.tensor.matmul(ap_ps[:], b_pa[:], ones_r[:, 0:L],
                         start=False, stop=True)
        ap_s = apool.tile([AD, L], f32, tag="ap_s", name="ap_s")
        nc.scalar.copy(ap_s[:], ap_ps[:])
        nc.sync.dma_start(o_ap[:], ap_s[:])

    nc.finalize()
    return nc


# =============================================================== marshaling
def _marshal_core(inputs, b, half):
    f = np.float16
    perm = (np.r_[HD:DI, 0:HD] if half == 1 else np.r_[0:DI])   # own half first

    in_w = inputs["in_w"]          # (NL, 2*DI, D)
    conv_w = inputs["conv_w"]      # (NL, DI, DC)
    inw_m = in_w[:, :DI, :][:, perm, :]                          # (NL, DI, D)
    conv_p = conv_w[:, perm, :]
    inw_r = np.ascontiguousarray(inw_m.transpose(0, 2, 1)).reshape(NL, KT, 128, DI)
    w_in = np.ascontiguousarray(inw_r.transpose(0, 2, 1, 3), dtype=f)
    w_cv = np.ascontiguousarray(
        conv_p.reshape(NL, FBLK, 128, DC).transpose(0, 2, 1, 3),
        dtype=np.float32)                                        # (NL,128,FBLK,DC)

    zrows = in_w[:, DI + half * HD: DI + (half + 1) * HD, :]     # (NL, HD, D)
    w_z = np.ascontiguousarray(
        zrows.transpose(0, 2, 1).reshape(NL, KT, 128, HD).transpose(0, 2, 1, 3),
        dtype=f)

    xp_w = inputs["xp_w"][:, :, perm]                            # (NL, RN, DI)
    w_xp = np.ascontiguousarray(
        xp_w.transpose(0, 2, 1).reshape(NL, FBLK, 128, RN).transpose(0, 2, 1, 3),
        dtype=f)

    own = slice(half * HD, (half + 1) * HD)
    w_dt = np.ascontiguousarray(
        inputs["dtp_w"][:, own, :].transpose(0, 2, 1), dtype=f)  # (NL, 32, HD)
    dtb = np.ascontiguousarray(
        inputs["dtp_b"][:, own].reshape(NL, NBLK, 128).transpose(2, 0, 1),
        dtype=np.float32)                                        # (128, NL, NBLK)
    A = -np.exp(inputs["A_log"].astype(np.float64)).astype(np.float32)
    asc = np.ascontiguousarray(
        A[:, own, :].reshape(NL, NBLK, 128, N).transpose(2, 0, 1, 3),
        dtype=np.float32)                                        # (128, NL, NBLK, N)
    dpr = np.ascontiguousarray(
        inputs["Dp"][:, own].reshape(NL, NBLK, 128).transpose(2, 0, 1),
        dtype=np.float32)
    b_cv = np.ascontiguousarray(
        inputs["conv_b"][:, perm].reshape(NL, FBLK, 128).transpose(0, 2, 1),
        dtype=f)

    out_w = inputs["out_w"]                                      # (NL, D, DI)
    w_out = np.ascontiguousarray(
        out_w.transpose(0, 2, 1).reshape(NL, FBLK, 128, D).transpose(0, 2, 1, 3),
        dtype=f)

    ln_w = np.concatenate([inputs["ln_w"], inputs["fn_w"][None]], 0)
    ln_b = np.concatenate([inputs["ln_b"], inputs["fn_b"][None]], 0)
    lnw = np.ascontiguousarray(
        ln_w.reshape(NL + 1, PBLK, 128).transpose(2, 0, 1), dtype=np.float32)
    lnb = np.ascontiguousarray(
        ln_b.reshape(NL + 1, PBLK, 128).transpose(2, 0, 1), dtype=np.float32)

    te = inputs["et_w"][inputs["timesteps"][b]]                  # (L, D)
    teT = np.ascontiguousarray(
        te.T.reshape(PBLK, 128, L).transpose(1, 0, 2), dtype=f)

    w_psh = np.ascontiguousarray(
        inputs["ps_w"].T.reshape(PBLK, 128, SD).transpose(1, 0, 2), dtype=f)
    w_pah = np.ascontiguousarray(
        inputs["pa_w"].T.reshape(PBLK, 128, AD).transpose(1, 0, 2), dtype=f)

    return {
        "i_sT": np.ascontiguousarray(inputs["states"][b].T, dtype=f),
        "i_aT": np.ascontiguousarray(inputs["actions"][b].T, dtype=f),
        "i_rcT": np.ascontiguousarray(
            np.concatenate([inputs["returns_to_go"][b].T,
                            inputs["constraints_to_go"][b].T], 1), dtype=f),
        "i_teT": teT,
        "i_wes": np.ascontiguousarray(inputs["es_w"].T, dtype=f),
        "i_wea": np.ascontiguousarray(inputs["ea_w"].T, dtype=f),
        "i_werc": np.ascontiguousarray(
            np.concatenate([inputs["er_w"].T, inputs["ec_w"].T], 1), dtype=f),
        "i_bemb": np.ascontiguousarray(
            np.concatenate([inputs["es_b"], inputs["ea_b"],
                            inputs["er_b"], inputs["ec_b"]])[None], dtype=f),
        "i_ones": np.ones((1, T), dtype=f),
        "i_onesp": np.ones((1, 128), dtype=f),
        "i_wstat": np.full((128, 1), 1.0 / D, dtype=f),
        "i_lnw": lnw, "i_lnb": lnb,
        "i_asc": asc, "i_dpr": dpr, "i_dtb": dtb,
        "i_wps": w_psh, "i_wpa": w_pah,
        "i_bps": np.ascontiguousarray(inputs["ps_b"][None], dtype=f),
        "i_bpa": np.ascontiguousarray(inputs["pa_b"][None], dtype=f),
        "i_win": w_in, "i_wcv": w_cv, "i_wz": w_z, "i_wxp": w_xp,
        "i_wdt": w_dt,
        "i_bcv": b_cv, "i_wout": w_out,
    }


def _check_dve_powers(inputs) -> bool:
    A = inputs["A_log"].astype(np.float64)
    if not np.allclose(A, A[:, :1, :], atol=1e-6):   # d-independence
        return False
    c = np.exp(A)
    for n, (a, b) in POW_PAIRS.items():
        if not np.allclose(c[..., n], c[..., a] + c[..., b], rtol=1e-4):
            return False
    return True


def marshal_inputs(inputs, n_pairs=B):
    inputs = {k: np.asarray(v) for k, v in inputs.items()}
    dve_ok = _check_dve_powers(inputs)
    in_maps = []
    for b in range(n_pairs):
        for half in (0, 1):
            in_maps.append(_marshal_core(inputs, b, half))
    return in_maps, dve_ok


# =============================================================== entry point
_CACHE = {}


def get_program(n_pairs, dve_ok=True):
    key = (n_pairs, dve_ok)
    if key not in _CACHE:
        _CACHE[key] = build_program(n_pairs, dve_powers=dve_ok)
    return _CACHE[key]


def kernel(**inputs):
    in_maps, dve_ok = marshal_inputs(inputs, n_pairs=B)
    nc = get_program(B, dve_ok)
    res = run_bass_kernel_spmd(nc, in_maps, list(range(2 * B)))
    sp = np.stack([res.results[2 * b]["o_spT"].T for b in range(B)], 0)
    ap = np.stack([res.results[2 * b]["o_apT"].T for b in range(B)], 0)
    return sp.astype(np.float32), ap.astype(np.float32)
